# revision 30
# baseline (speedup 1.0000x reference)
"""AffinityHead Trainium2 kernel (v3: interleaved conv+affinity).

Reference computation:
  f = ELU(concat(w83@conv4, w84@conv5, w85@conv6))   (1x1 convs, per pixel)
  x = ELU(w9 @ f)                                     [B, 448, 56, 56]
  aff[b,d,p] = exp(-mean_c |x[c, to(d,p)] - x[c, from(p)]|)   [B, 34, 2496]

Sharding: 8 cores = 4 images x 2 row-halves. Each core handles 26 from-rows
(+4 halo rows) = 30 rows of one image; SPMD identical program.

Design (measured 250us/core; v1 serial conv-then-affinity was 299-380us):
- x stored as [128, 4, NPX] bf16 (448 ch padded to 4x128; pad rows zero) so
  one TT subtract + one int16-mask abs covers all channels per offset.
- inputs staged per SLAB in uniform ~430KB cast-DMA transfers (c6 split
  16-way) so slabs land in order every ~26us; a single 860KB transfer
  alone on one DMA engine would take ~38us and delay the first band.
- affinity emitted in 3 row-bands; each band's triples are pumped between
  conv matmul sections of the following slab so DVE/ACT/PE interleave conv
  and affinity work instead of phase-serializing.
- channel reduce: ones-matmuls (K=128 incl. zero pad) with 3 offsets per
  PSUM bank at partition bases {0,32,64} (PE col-tile 32); exp batches 3
  offsets in one contiguous-partition ACT op; per-row out DMAs.
- ELU = max(p, exp(-relu(-p))-1): Relu+Exp on ACT (one shared act table
  with Exp/Abs/Copy: no table reloads), single STT on DVE.
- abs split ~17/17 between DVE int16-mask (4x mode) and ACT Abs to equalize
  engine totals (DVE ~150us, ACT ~155us, PE ~162us active).

Hard-won stack constraints (this container's walrus):
- build on bacc.Bacc and call nc.finalize() (sync legalization).
- matmul/AP base partition must be 0, 32, or 64 (96 rejected).
- BIR verifier rejects partition-step APs on engine args (DMA is fine).
- InstMatmult.ldweights=False corrupts results even between back-to-back
  same-weight matmuls on the same PE col-tile: do not use.
- only gpsimd can issue casting DMAs (SWDGE); anything else queued on
  gpsimd serializes behind staging issues (their tile-WAR waits block the
  queue), so band work stays off gpsimd entirely.
- Pool engine lacks int16 bitwise ops (no mask-abs on gpsimd).
- keep DMAs contiguous/few; never read SBUF written later in program order.
- DVE 2x mode needs 2-byte dtype + stride-1 innermost + 4B-aligned start
  (xo = odd-shifted copy of xg serves odd-dx to_views); fp8 anywhere in a
  DVE op drops it to 1x, which is why fp8/DoubleRow reduction loses.

Tuning notes: stage6 bufs=32 (2 slabs) required - fewer serializes
stage->compute; dpool 12 / apool 6 / psA 2; pump rates 2-2-2-2/1/1/1x4
per slab. Next levers: fp8 conv6 matmuls (PE -40us, rel err ~2-3x),
finer slabs to pull band starts earlier (costs LDWEIGHTS), trimming the
~15us startup semaphore storm.
"""
import numpy as np
from contextlib import ExitStack

import concourse.bass as bass
from concourse import bacc
import concourse.mybir as mybir
import concourse.tile as tile
from concourse.bass_utils import run_bass_kernel_spmd

RAD = 5
W = 56
ROWS = 30            # rows of x per core (26 from + 4 halo)
FROM_ROWS = 26
NPX = ROWS * W       # 1680
NPAIR = FROM_ROWS * 48   # 1248
C = 448
N_CORES = 8

F32 = mybir.dt.float32
BF16 = mybir.dt.bfloat16

USE_LDW_SKIP = False   # ldweights=False on repeated ones-matmuls


def _offsets():
    out = []
    for x in range(1, RAD):
        out.append((0, x))
    for y in range(1, RAD):
        for x in range(-RAD + 1, RAD):
            if x * x + y * y < RAD * RAD:
                out.append((y, x))
    return out


OFFS = _offsets()            # 34 (dy, dx), matching reference search_dist order
assert len(OFFS) == 34

# w9 contraction split aligned to feature-group boundaries (f83|f84|f85a|f85b)
KSPLIT = [(0, 64), (64, 128), (192, 128), (320, 128)]
# x output channel groups: 4 groups of <=128 (padded to 128 in storage)
MSPLIT = [(0, 128), (128, 128), (256, 128), (384, 64)]

SLAB = 420                   # pixel slab for PSUM-resident f/x (1 bank)
NSLAB = NPX // SLAB          # 4

# affinity bands: (from_row0, nrows, emit_after_slab_index)
# band needs x px <= (r0+nr+3)*56-1 (+1 for xo) covered by slabs 0..idx
BANDS = [(0, 10, 1), (10, 8, 2), (18, 8, 3)]
for _r0, _nr, _si in BANDS:
    # last to-row is r0+nr-1+4; its last pixel (col 55) must be in-slab
    assert (_r0 + _nr + 4) * W <= (_si + 1) * SLAB, (_r0, _nr, _si)

# engine assignment per offset index: subtract and abs
# 'v' = DVE, 'a' = ACT, 'g' = GPSIMD
SUB_ENGINE = ['v'] * 34   # gpsimd queue is poisoned by staging DMA issues
# abs: ACT Abs costs ~3.2x DVE's int16-mask per offset, but DVE is the
# critical engine; equalizing engine totals puts ~22 of 34 on ACT.
# (Abs shares the Exp act table: no table reloads.)
ABS_ENGINE = ['a' if _d % 2 else 'v' for _d in range(34)]


def _emit(ctx: ExitStack, tc: "tile.TileContext", io: dict):
    nc = tc.nc
    c6, c5, c4 = io["c6"], io["c5"], io["c4"]
    out_d = io["out"]

    persist = ctx.enter_context(tc.tile_pool(name="persist", bufs=1))
    stage6 = ctx.enter_context(tc.tile_pool(name="stage6", bufs=32))
    stage5 = ctx.enter_context(tc.tile_pool(name="stage5", bufs=8))
    stage4 = ctx.enter_context(tc.tile_pool(name="stage4", bufs=4))
    fpool = ctx.enter_context(tc.tile_pool(name="fpool", bufs=3))
    tpool = ctx.enter_context(tc.tile_pool(name="tmp", bufs=4))
    dpool = ctx.enter_context(tc.tile_pool(name="dtv", bufs=12))
    apool = ctx.enter_context(tc.tile_pool(name="aff", bufs=6))
    psF = ctx.enter_context(tc.tile_pool(name="psF", bufs=1, space="PSUM"))
    psX = ctx.enter_context(tc.tile_pool(name="psX", bufs=2, space="PSUM"))
    psA = ctx.enter_context(tc.tile_pool(name="psA", bufs=2, space="PSUM"))

    # ---- weights into SBUF: ONE packed f32r DMA + ONE packed bf16 DMA ----
    wcs = persist.tile([128, 9472], BF16, name="wcs")
    nc.sync.dma_start(wcs[:], io["wc"][:])
    w9cs = persist.tile([128, 4, 448], BF16, name="w9cs")
    nc.sync.dma_start(w9cs[:], io["w9c"][:].rearrange("p (k m) -> p k m", k=4))

    def w85_sl(kt, m):
        base = kt * 256 + m * 128
        return wcs[:, base:base + 128]

    def w84_sl(kt):
        return wcs[:, 8192 + kt * 128:8192 + (kt + 1) * 128]

    def w83_sl(kt):
        return wcs[:, 9216 + kt * 64:9216 + (kt + 1) * 64]

    ones = persist.tile([128, 1], BF16, name="ones")
    nc.vector.memset(ones[:], 1.0)
    mask16 = persist.tile([128, 1920], mybir.dt.int16, name="mask16")
    nc.vector.memset(mask16[:], 32767)   # 0x7fff: clears bf16 sign bit

    # ---- x storage (bf16, 4x128 padded groups) + odd-shifted copy ----
    xg = persist.tile([128, 4, NPX], BF16, name="xg", tag="xg")
    xo = persist.tile([128, 4, NPX], BF16, name="xo", tag="xo")
    # zero the pad rows of group 3 (channels 448..511); elu writes 0:64 only
    nc.vector.memset(xg[64:128, 3, :], 0.0)
    nc.vector.memset(xo[64:128, 3, :], 0.0)

    # ---- ELU helper: out = max(p, exp(min(p,0)) - 1), p in PSUM.
    # min(p,0) = -relu(-p) on ACT (Relu/Exp share one act table) so the DVE
    # pays only one 1x STT instead of TS-min + STT (DVE is the critical
    # engine; ACT has headroom).
    def elu(psrc, dst, pn, fn):
        r = tpool.tile([pn, fn], BF16, tag="elu_m", name="elu_m")
        nc.scalar.activation(out=r[:], in_=psrc, scale=-1.0,
                             func=mybir.ActivationFunctionType.Relu)
        e = tpool.tile([pn, fn], BF16, tag="elu_e", name="elu_e")
        nc.scalar.activation(out=e[:], in_=r[:], scale=-1.0,
                             func=mybir.ActivationFunctionType.Exp)
        nc.vector.scalar_tensor_tensor(
            out=dst, in0=e[:], scalar=-1.0, in1=psrc,
            op0=mybir.AluOpType.add, op1=mybir.AluOpType.max)

    # ---- conv input staging: cast-DMA (fp32 HBM -> bf16 SBUF).
    # Staged per SLAB in uniform ~430KB transfers (c6 split 16-way) so one
    # slab's full contraction dim lands in ~25us and slabs complete in
    # order; a single 860KB transfer alone on one DMA engine takes ~38us.
    # Cast DMAs must be SWDGE (gpsimd-issued); band work therefore avoids
    # the gpsimd queue entirely (it would serialize behind staging issues).
    def stage_slab(dram, n_super, ktile_per, s, pool):
        tiles = []
        for skt in range(n_super):
            t = pool.tile([128, ktile_per, SLAB], BF16, tag="cst", name="cst")
            view = dram[:].rearrange("(s k p) n -> s p k n", k=ktile_per, p=128)
            nc.gpsimd.dma_start(t[:], view[skt, :, :, s * SLAB:(s + 1) * SLAB])
            tiles.append(t)
        return tiles

    cslab = []
    for s in range(NSLAB):
        cslab.append({
            "c6": stage_slab(c6, 16, 2, s, stage6),
            "c5": stage_slab(c5, 4, 2, s, stage5),
            "c4": stage_slab(c4, 2, 2, s, stage4),
        })

    xg_r = xg[:].rearrange("p g (r c) -> p g r c", c=W)
    xo_r = xo[:].rearrange("p g (r c) -> p g r c", c=W)

    # Band triples are emitted as a generator and "pumped" between conv
    # matmul sections, so band work interleaves into the PE/DVE/ACT queues
    # as soon as its x rows exist instead of bunching after each slab.
    def emit_band(r0, nr):
        npair = nr * 48
        for t3 in range(12):
            k = min(3, 34 - t3 * 3)
            pst = psA.tile([128, 512], F32, tag="pst", name="pst")
            arow = apool.tile([32 * (k - 1) + 1, npair], F32, tag="arow",
                              name="arow")
            for j in range(k):
                d_idx = t3 * 3 + j
                dy, dx = OFFS[d_idx]
                dt = dpool.tile([128, 4, nr, 48], BF16, tag="dt", name="dt")
                if dx % 2 == 0:
                    src, cof = xg_r, dx
                else:
                    src, cof = xo_r, dx - 1
                to_view = src[:, :, r0 + dy:r0 + dy + nr, 4 + cof:52 + cof]
                from_view = xg_r[:, :, r0:r0 + nr, 4:52]
                se = SUB_ENGINE[d_idx]
                if se == 'g':
                    nc.gpsimd.tensor_tensor(out=dt[:], in0=to_view,
                                            in1=from_view,
                                            op=mybir.AluOpType.subtract)
                else:
                    nc.vector.tensor_tensor(out=dt[:], in0=to_view,
                                            in1=from_view,
                                            op=mybir.AluOpType.subtract)
                dflat = dt[:].rearrange("p g r c -> p (g r c)")
                ae = ABS_ENGINE[d_idx]
                if ae == 'a':
                    nc.scalar.activation(out=dflat, in_=dflat,
                                         func=mybir.ActivationFunctionType.Abs)
                elif ae == 'g':
                    di = dflat.bitcast(mybir.dt.int16)
                    nc.gpsimd.tensor_tensor(out=di, in0=di,
                                            in1=mask16[:, :4 * npair],
                                            op=mybir.AluOpType.bitwise_and)
                else:
                    di = dflat.bitcast(mybir.dt.int16)
                    nc.vector.tensor_scalar(out=di, in0=di, scalar1=32767,
                                            scalar2=None,
                                            op0=mybir.AluOpType.bitwise_and)
                for g in range(4):
                    mm = nc.tensor.matmul(
                        pst[32 * j:32 * j + 1, :npair], ones[:],
                        dflat[:, g * npair:(g + 1) * npair],
                        start=(g == 0), stop=(g == 3))
                    # weight reuse is only reliable between back-to-back
                    # matmuls on the same PE col-tile with nothing between
                    if USE_LDW_SKIP and g > 0:
                        mm.ldweights = False
            # exp over partitions 0..(32k-31): covers bases {0,32,64} plus
            # don't-care rows (cost is free-size cycles; partitions parallel).
            # Strided-partition APs are rejected by the BIR verifier, so read
            # contiguously and DMA the 3 real rows individually.
            pn = 32 * (k - 1) + 1
            nc.scalar.activation(out=arow[:], in_=pst[0:pn, :npair],
                                 func=mybir.ActivationFunctionType.Exp,
                                 scale=-1.0 / C)
            for j in range(k):
                d_out = t3 * 3 + j
                nc.sync.dma_start(
                    out_d[d_out:d_out + 1, r0 * 48:(r0 + nr) * 48],
                    arow[32 * j:32 * j + 1, :])
            yield

    # ---- conv + x phase, slab by slab, band triples pumped between
    # conv sections ----
    band_q = list(BANDS)
    active = []   # generators of ready bands

    def pump(n):
        while n > 0 and active:
            try:
                next(active[0])
                n -= 1
            except StopIteration:
                active.pop(0)

    for s in range(NSLAB):
        s0 = s * SLAB
        hs = cslab[s]

        def load(dram_tiles, kt):
            return dram_tiles[kt // 2][:, kt % 2, :]

        # f85: 2 M-tiles of 128 out-ch; band triples pumped every 8 ktiles
        f85p = [psF.tile([128, SLAB], F32, tag=f"f85{m}", name=f"f85p{m}") for m in range(2)]
        for kt in range(32):
            rhs = load(hs["c6"], kt)
            for m in range(2):
                nc.tensor.matmul(
                    f85p[m][:], w85_sl(kt, m),
                    rhs, start=(kt == 0), stop=(kt == 31))
            if kt % 8 == 7:
                pump(2)
        f84p = psF.tile([128, SLAB], F32, tag="f84", name="f84p")
        for kt in range(8):
            nc.tensor.matmul(f84p[:], w84_sl(kt), load(hs["c5"], kt),
                             start=(kt == 0), stop=(kt == 7))
        pump(1)
        f83p = psF.tile([64, SLAB], F32, tag="f83", name="f83p")
        for kt in range(4):
            nc.tensor.matmul(f83p[:], w83_sl(kt), load(hs["c4"], kt),
                             start=(kt == 0), stop=(kt == 3))
        pump(1)

        # ELU f -> sbuf k-group tiles (64/128/128/128 partitions)
        fk = [fpool.tile([kn, SLAB], BF16, tag=f"fk{i}", name=f"fk{i}")
              for i, (k0, kn) in enumerate(KSPLIT)]
        elu(f83p[:], fk[0][:], 64, SLAB)
        elu(f84p[:], fk[1][:], 128, SLAB)
        elu(f85p[0][:], fk[2][:], 128, SLAB)
        elu(f85p[1][:], fk[3][:], 128, SLAB)

        # x = ELU(w9 @ f): M-tiles sequential to cap PSUM use
        sl = slice(s0, s0 + SLAB)
        for mt, (m0, mn) in enumerate(MSPLIT):
            xp = psX.tile([mn, SLAB], F32, tag="xp", name="xp")
            for kt in range(4):
                nc.tensor.matmul(xp[:], w9cs[0:KSPLIT[kt][1], kt, m0:m0 + mn],
                                 fk[kt][:], start=(kt == 0), stop=(kt == 3))
            elu(xp[:], xg[0:mn, mt, sl], mn, SLAB)
            pump(1)
        # odd-shifted copy, reading only already-written xg
        start = s0 - 1 if s > 0 else 0
        nc.vector.tensor_copy(out=xo[:, :, start:s0 + SLAB - 1],
                              in_=xg[:, :, start + 1:s0 + SLAB])

        # activate any bands whose x pixels are now all written
        while band_q and band_q[0][2] == s:
            r0, nr, _ = band_q.pop(0)
            active.append(emit_band(r0, nr))

    # drain remaining band work (last band has no following slab)
    pump(10 ** 6)


_NC_CACHE = {}
LAST_RESULT = None


def _build_nc():
    if "nc" in _NC_CACHE:
        return _NC_CACHE["nc"]
    nc = bacc.Bacc()
    io = {
        "c6": nc.declare_dram_parameter("c6", [4096, NPX], F32, isOutput=False),
        "c5": nc.declare_dram_parameter("c5", [1024, NPX], F32, isOutput=False),
        "c4": nc.declare_dram_parameter("c4", [512, NPX], F32, isOutput=False),
        "wc": nc.declare_dram_parameter("wc", [128, 9472], BF16, isOutput=False),
        "w9c": nc.declare_dram_parameter("w9c", [128, 4 * 448], BF16, isOutput=False),
        "out": nc.declare_dram_parameter("out", [34, NPAIR], F32, isOutput=True),
    }
    with tile.TileContext(nc) as tc:
        with ExitStack() as ctx:
            _emit(ctx, tc, io)
    nc.finalize()   # Bacc.compile(): 1-wait legalization + event semaphores
    _NC_CACHE["nc"] = nc
    return nc


def _expected_indices():
    full = np.reshape(np.arange(0, 56 * 56, dtype=np.int64), (56, 56))
    ind_from = np.reshape(full[:-4, 4:-4], [-1])
    tos = []
    for dy, dx in OFFS:
        tos.append(np.reshape(full[dy:dy + 52, 4 + dx:4 + dx + 48], [-1]))
    return ind_from, np.concatenate(tos, axis=0)


def _maybe_install_trace_hook():
    import os
    if not os.environ.get("BASS_TRACE"):
        return
    import sys
    import types
    try:
        import antenv.axon_hooks  # noqa: F401
        return
    except ImportError:
        pass
    try:
        from trn_agent_boot.trn_boot import _ntff_profile_via_ctypes
        hook = _ntff_profile_via_ctypes('/opt/axon/libaxon_pjrt.so')
    except Exception:
        hook = None
    import antenv
    mod = types.ModuleType("antenv.axon_hooks")
    mod.get_axon_ntff_profile_hook = lambda: hook
    mod.set_axon_ntff_profile_hook = lambda h: None
    sys.modules["antenv.axon_hooks"] = mod
    antenv.axon_hooks = mod


def kernel(conv4, conv5, conv6, w83, w84, w85, w9, ind_from, ind_to):
    import ml_dtypes
    conv4 = np.asarray(conv4, dtype=np.float32)
    conv5 = np.asarray(conv5, dtype=np.float32)
    conv6 = np.asarray(conv6, dtype=np.float32)
    ef, et = _expected_indices()
    assert np.array_equal(np.asarray(ind_from), ef), "unexpected ind_from"
    assert np.array_equal(np.asarray(ind_to), et), "unexpected ind_to"

    def warrange(w, ktiles):
        # w [M, K] -> lhsT tiles layout [128, ktiles*M]
        wt = np.asarray(w, np.float32).T            # [K, M]
        K, M = wt.shape
        return np.ascontiguousarray(
            wt.reshape(ktiles, 128, M).transpose(1, 0, 2).reshape(128, ktiles * M))

    wc = np.ascontiguousarray(np.concatenate(
        [warrange(w85, 32), warrange(w84, 8), warrange(w83, 4)],
        axis=1)).astype(ml_dtypes.bfloat16)
    w9t_f = np.asarray(w9, np.float32).T          # [448 in, 448 out]
    w9c = np.zeros((128, 4, 448), np.float32)
    ks = [(0, 64), (64, 128), (192, 128), (320, 128)]
    for i, (k0, kn) in enumerate(ks):
        w9c[0:kn, i, :] = w9t_f[k0:k0 + kn, :]
    w9c = np.ascontiguousarray(w9c.reshape(128, 4 * 448)).astype(
        ml_dtypes.bfloat16)

    in_maps = []
    for core in range(N_CORES):
        b, half = core // 2, core % 2
        r0 = 0 if half == 0 else 26
        in_maps.append({
            "c6": np.ascontiguousarray(
                conv6[b, :, r0:r0 + ROWS, :].reshape(4096, NPX)),
            "c5": np.ascontiguousarray(
                conv5[b, :, r0:r0 + ROWS, :].reshape(1024, NPX)),
            "c4": np.ascontiguousarray(
                conv4[b, :, r0:r0 + ROWS, :].reshape(512, NPX)),
            "wc": wc, "w9c": w9c,
        })

    _maybe_install_trace_hook()
    nc = _build_nc()
    res = run_bass_kernel_spmd(nc, in_maps, list(range(N_CORES)))
    global LAST_RESULT
    LAST_RESULT = res

    aff = np.empty((4, 34, 2496), np.float32)
    for core in range(N_CORES):
        b, half = core // 2, core % 2
        aff[b, :, half * NPAIR:(half + 1) * NPAIR] = res.results[core]["out"]
    return aff


# revision 31
# speedup vs baseline: 1.0824x; 1.0824x over previous
"""AffinityHead Trainium2 kernel (v3: interleaved conv+affinity).

Reference computation:
  f = ELU(concat(w83@conv4, w84@conv5, w85@conv6))   (1x1 convs, per pixel)
  x = ELU(w9 @ f)                                     [B, 448, 56, 56]
  aff[b,d,p] = exp(-mean_c |x[c, to(d,p)] - x[c, from(p)]|)   [B, 34, 2496]

Sharding: 8 cores = 4 images x 2 row-halves. Each core handles 26 from-rows
(+4 halo rows) = 30 rows of one image; SPMD identical program.

Design (measured 250us/core; v1 serial conv-then-affinity was 299-380us):
- x stored as [128, 4, NPX] bf16 (448 ch padded to 4x128; pad rows zero) so
  one TT subtract + one int16-mask abs covers all channels per offset.
- inputs staged per SLAB in uniform ~430KB cast-DMA transfers (c6 split
  16-way) so slabs land in order every ~26us; a single 860KB transfer
  alone on one DMA engine would take ~38us and delay the first band.
- affinity emitted in 3 row-bands; each band's triples are pumped between
  conv matmul sections of the following slab so DVE/ACT/PE interleave conv
  and affinity work instead of phase-serializing.
- channel reduce: ones-matmuls (K=128 incl. zero pad) with 3 offsets per
  PSUM bank at partition bases {0,32,64} (PE col-tile 32); exp batches 3
  offsets in one contiguous-partition ACT op; per-row out DMAs.
- ELU = max(p, exp(-relu(-p))-1): Relu+Exp on ACT (one shared act table
  with Exp/Abs/Copy: no table reloads), single STT on DVE.
- abs split ~17/17 between DVE int16-mask (4x mode) and ACT Abs to equalize
  engine totals (DVE ~150us, ACT ~155us, PE ~162us active).

Hard-won stack constraints (this container's walrus):
- build on bacc.Bacc and call nc.finalize() (sync legalization).
- matmul/AP base partition must be 0, 32, or 64 (96 rejected).
- BIR verifier rejects partition-step APs on engine args (DMA is fine).
- InstMatmult.ldweights=False corrupts results even between back-to-back
  same-weight matmuls on the same PE col-tile: do not use.
- only gpsimd can issue casting DMAs (SWDGE); anything else queued on
  gpsimd serializes behind staging issues (their tile-WAR waits block the
  queue), so band work stays off gpsimd entirely.
- Pool engine lacks int16 bitwise ops (no mask-abs on gpsimd).
- keep DMAs contiguous/few; never read SBUF written later in program order.
- DVE 2x mode needs 2-byte dtype + stride-1 innermost + 4B-aligned start
  (xo = odd-shifted copy of xg serves odd-dx to_views); fp8 anywhere in a
  DVE op drops it to 1x, which is why fp8/DoubleRow reduction loses.

Tuning notes: stage6 bufs=32 (2 slabs) required - fewer serializes
stage->compute; dpool 12 / apool 6 / psA 2; pump rates 2-2-2-2/1/1/1x4
per slab. Next levers: fp8 conv6 matmuls (PE -40us, rel err ~2-3x),
finer slabs to pull band starts earlier (costs LDWEIGHTS), trimming the
~15us startup semaphore storm.
"""
import numpy as np
from contextlib import ExitStack

import concourse.bass as bass
from concourse import bacc
import concourse.mybir as mybir
import concourse.tile as tile
from concourse.bass_utils import run_bass_kernel_spmd

RAD = 5
W = 56
ROWS = 30            # rows of x per core (26 from + 4 halo)
FROM_ROWS = 26
NPX = ROWS * W       # 1680
NPAIR = FROM_ROWS * 48   # 1248
C = 448
N_CORES = 8

F32 = mybir.dt.float32
BF16 = mybir.dt.bfloat16

USE_LDW_SKIP = False   # ldweights=False on repeated ones-matmuls


def _offsets():
    out = []
    for x in range(1, RAD):
        out.append((0, x))
    for y in range(1, RAD):
        for x in range(-RAD + 1, RAD):
            if x * x + y * y < RAD * RAD:
                out.append((y, x))
    return out


OFFS = _offsets()            # 34 (dy, dx), matching reference search_dist order
assert len(OFFS) == 34

# w9 contraction split aligned to feature-group boundaries (f83|f84|f85a|f85b)
KSPLIT = [(0, 64), (64, 128), (192, 128), (320, 128)]
# x output channel groups: 4 groups of <=128 (padded to 128 in storage)
MSPLIT = [(0, 128), (128, 128), (256, 128), (384, 64)]

SLAB = 420                   # pixel slab for PSUM-resident f/x (1 bank)
NSLAB = NPX // SLAB          # 4

# affinity bands: (from_row0, nrows, emit_after_slab_index)
# band needs x px <= (r0+nr+3)*56-1 (+1 for xo) covered by slabs 0..idx
BANDS = [(0, 10, 1), (10, 8, 2), (18, 8, 3)]
for _r0, _nr, _si in BANDS:
    # last to-row is r0+nr-1+4; its last pixel (col 55) must be in-slab
    assert (_r0 + _nr + 4) * W <= (_si + 1) * SLAB, (_r0, _nr, _si)

# engine assignment per offset index: subtract and abs
# 'v' = DVE, 'a' = ACT, 'g' = GPSIMD
SUB_ENGINE = ['v'] * 34   # gpsimd queue is poisoned by staging DMA issues
# abs: ACT Abs costs ~3.2x DVE's int16-mask per offset, but DVE is the
# critical engine; equalizing engine totals puts ~22 of 34 on ACT.
# (Abs shares the Exp act table: no table reloads.)
ABS_ENGINE = ['a' if _d % 2 else 'v' for _d in range(34)]


def _emit(ctx: ExitStack, tc: "tile.TileContext", io: dict):
    nc = tc.nc
    c6, c5, c4 = io["c6"], io["c5"], io["c4"]
    out_d = io["out"]

    persist = ctx.enter_context(tc.tile_pool(name="persist", bufs=1))
    stage6 = ctx.enter_context(tc.tile_pool(name="stage6", bufs=32))
    stage5 = ctx.enter_context(tc.tile_pool(name="stage5", bufs=8))
    stage4 = ctx.enter_context(tc.tile_pool(name="stage4", bufs=4))
    fpool = ctx.enter_context(tc.tile_pool(name="fpool", bufs=3))
    tpool = ctx.enter_context(tc.tile_pool(name="tmp", bufs=4))
    dpool = ctx.enter_context(tc.tile_pool(name="dtv", bufs=12))
    apool = ctx.enter_context(tc.tile_pool(name="aff", bufs=6))
    psF = ctx.enter_context(tc.tile_pool(name="psF", bufs=1, space="PSUM"))
    psX = ctx.enter_context(tc.tile_pool(name="psX", bufs=1, space="PSUM"))
    psA = ctx.enter_context(tc.tile_pool(name="psA", bufs=3, space="PSUM"))

    # ---- weights into SBUF: ONE packed f32r DMA + ONE packed bf16 DMA ----
    wcs = persist.tile([128, 9472], BF16, name="wcs")
    nc.sync.dma_start(wcs[:], io["wc"][:])
    w9cs = persist.tile([128, 4, 448], BF16, name="w9cs")
    nc.sync.dma_start(w9cs[:], io["w9c"][:].rearrange("p (k m) -> p k m", k=4))

    def w85_sl(kt, m):
        base = kt * 256 + m * 128
        return wcs[:, base:base + 128]

    def w84_sl(kt):
        return wcs[:, 8192 + kt * 128:8192 + (kt + 1) * 128]

    def w83_sl(kt):
        return wcs[:, 9216 + kt * 64:9216 + (kt + 1) * 64]

    ones = persist.tile([128, 1], BF16, name="ones")
    nc.vector.memset(ones[:], 1.0)
    mask16 = persist.tile([128, 1920], mybir.dt.int16, name="mask16")
    nc.vector.memset(mask16[:], 32767)   # 0x7fff: clears bf16 sign bit

    # ---- x storage (bf16, 4x128 padded groups) + odd-shifted copy ----
    xg = persist.tile([128, 4, NPX], BF16, name="xg", tag="xg")
    xo = persist.tile([128, 4, NPX], BF16, name="xo", tag="xo")
    # zero the pad rows of group 3 (channels 448..511); elu writes 0:64 only
    nc.vector.memset(xg[64:128, 3, :], 0.0)
    nc.vector.memset(xo[64:128, 3, :], 0.0)

    # ---- ELU helper: out = max(p, exp(min(p,0)) - 1), p in PSUM.
    # min(p,0) = -relu(-p) on ACT (Relu/Exp share one act table) so the DVE
    # pays only one 1x STT instead of TS-min + STT (DVE is the critical
    # engine; ACT has headroom).
    def elu(psrc, dst, pn, fn):
        r = tpool.tile([pn, fn], BF16, tag="elu_m", name="elu_m")
        nc.scalar.activation(out=r[:], in_=psrc, scale=-1.0,
                             func=mybir.ActivationFunctionType.Relu)
        e = tpool.tile([pn, fn], BF16, tag="elu_e", name="elu_e")
        nc.scalar.activation(out=e[:], in_=r[:], scale=-1.0,
                             func=mybir.ActivationFunctionType.Exp)
        nc.vector.scalar_tensor_tensor(
            out=dst, in0=e[:], scalar=-1.0, in1=psrc,
            op0=mybir.AluOpType.add, op1=mybir.AluOpType.max)

    # ---- conv input staging: cast-DMA (fp32 HBM -> bf16 SBUF).
    # Staged per SLAB in uniform ~430KB transfers (c6 split 16-way) so one
    # slab's full contraction dim lands in ~25us and slabs complete in
    # order; a single 860KB transfer alone on one DMA engine takes ~38us.
    # Cast DMAs must be SWDGE (gpsimd-issued); band work therefore avoids
    # the gpsimd queue entirely (it would serialize behind staging issues).
    def stage_slab(dram, n_super, ktile_per, s, pool):
        tiles = []
        for skt in range(n_super):
            t = pool.tile([128, ktile_per, SLAB], BF16, tag="cst", name="cst")
            view = dram[:].rearrange("(s k p) n -> s p k n", k=ktile_per, p=128)
            nc.gpsimd.dma_start(t[:], view[skt, :, :, s * SLAB:(s + 1) * SLAB])
            tiles.append(t)
        return tiles

    cslab = []
    for s in range(NSLAB):
        # c5/c4 first: f84/f83 + their fk ELUs then run while c6 is still
        # streaming, shortening the serial chain from last-c6-tile to x
        sl5 = stage_slab(c5, 4, 2, s, stage5)
        sl4 = stage_slab(c4, 2, 2, s, stage4)
        cslab.append({
            "c6": stage_slab(c6, 16, 2, s, stage6),
            "c5": sl5,
            "c4": sl4,
        })

    xg_r = xg[:].rearrange("p g (r c) -> p g r c", c=W)
    xo_r = xo[:].rearrange("p g (r c) -> p g r c", c=W)

    # Band triples are emitted as a generator and "pumped" between conv
    # matmul sections, so band work interleaves into the PE/DVE/ACT queues
    # as soon as its x rows exist instead of bunching after each slab.
    def emit_band(r0, nr):
        npair = nr * 48
        for t3 in range(12):
            k = min(3, 34 - t3 * 3)
            pst = psA.tile([128, 512], F32, tag="pst", name="pst")
            arow = apool.tile([32 * (k - 1) + 1, npair], F32, tag="arow",
                              name="arow")
            for j in range(k):
                d_idx = t3 * 3 + j
                dy, dx = OFFS[d_idx]
                dt = dpool.tile([128, 4, nr, 48], BF16, tag="dt", name="dt")
                if dx % 2 == 0:
                    src, cof = xg_r, dx
                else:
                    src, cof = xo_r, dx - 1
                to_view = src[:, :, r0 + dy:r0 + dy + nr, 4 + cof:52 + cof]
                from_view = xg_r[:, :, r0:r0 + nr, 4:52]
                se = SUB_ENGINE[d_idx]
                if se == 'g':
                    nc.gpsimd.tensor_tensor(out=dt[:], in0=to_view,
                                            in1=from_view,
                                            op=mybir.AluOpType.subtract)
                else:
                    nc.vector.tensor_tensor(out=dt[:], in0=to_view,
                                            in1=from_view,
                                            op=mybir.AluOpType.subtract)
                dflat = dt[:].rearrange("p g r c -> p (g r c)")
                ae = ABS_ENGINE[d_idx]
                if ae == 'a':
                    nc.scalar.activation(out=dflat, in_=dflat,
                                         func=mybir.ActivationFunctionType.Abs)
                elif ae == 'g':
                    di = dflat.bitcast(mybir.dt.int16)
                    nc.gpsimd.tensor_tensor(out=di, in0=di,
                                            in1=mask16[:, :4 * npair],
                                            op=mybir.AluOpType.bitwise_and)
                else:
                    di = dflat.bitcast(mybir.dt.int16)
                    nc.vector.tensor_scalar(out=di, in0=di, scalar1=32767,
                                            scalar2=None,
                                            op0=mybir.AluOpType.bitwise_and)
                for g in range(4):
                    mm = nc.tensor.matmul(
                        pst[32 * j:32 * j + 1, :npair], ones[:],
                        dflat[:, g * npair:(g + 1) * npair],
                        start=(g == 0), stop=(g == 3))
                    # weight reuse is only reliable between back-to-back
                    # matmuls on the same PE col-tile with nothing between
                    if USE_LDW_SKIP and g > 0:
                        mm.ldweights = False
            # exp over partitions 0..(32k-31): covers bases {0,32,64} plus
            # don't-care rows (cost is free-size cycles; partitions parallel).
            # Strided-partition APs are rejected by the BIR verifier, so read
            # contiguously and DMA the 3 real rows individually.
            pn = 32 * (k - 1) + 1
            nc.scalar.activation(out=arow[:], in_=pst[0:pn, :npair],
                                 func=mybir.ActivationFunctionType.Exp,
                                 scale=-1.0 / C)
            for j in range(k):
                d_out = t3 * 3 + j
                nc.sync.dma_start(
                    out_d[d_out:d_out + 1, r0 * 48:(r0 + nr) * 48],
                    arow[32 * j:32 * j + 1, :])
            yield

    # ---- conv + x phase, slab by slab, band triples pumped between
    # conv sections ----
    band_q = list(BANDS)
    active = []   # generators of ready bands

    def pump(n):
        while n > 0 and active:
            try:
                next(active[0])
                n -= 1
            except StopIteration:
                active.pop(0)

    for s in range(NSLAB):
        s0 = s * SLAB
        hs = cslab[s]

        def load(dram_tiles, kt):
            return dram_tiles[kt // 2][:, kt % 2, :]

        # f85: 2 M-tiles of 128 out-ch; band triples pumped every 8 ktiles
        f85p = [psF.tile([128, SLAB], F32, tag=f"f85{m}", name=f"f85p{m}") for m in range(2)]
        for kt in range(32):
            rhs = load(hs["c6"], kt)
            for m in range(2):
                nc.tensor.matmul(
                    f85p[m][:], w85_sl(kt, m),
                    rhs, start=(kt == 0), stop=(kt == 31))
            if kt % 8 == 7:
                pump(2)
        f84p = psF.tile([128, SLAB], F32, tag="f84", name="f84p")
        for kt in range(8):
            nc.tensor.matmul(f84p[:], w84_sl(kt), load(hs["c5"], kt),
                             start=(kt == 0), stop=(kt == 7))
        pump(1)
        f83p = psF.tile([64, SLAB], F32, tag="f83", name="f83p")
        for kt in range(4):
            nc.tensor.matmul(f83p[:], w83_sl(kt), load(hs["c4"], kt),
                             start=(kt == 0), stop=(kt == 3))
        pump(1)

        # ELU f -> sbuf k-group tiles (64/128/128/128 partitions)
        fk = [fpool.tile([kn, SLAB], BF16, tag=f"fk{i}", name=f"fk{i}")
              for i, (k0, kn) in enumerate(KSPLIT)]
        elu(f83p[:], fk[0][:], 64, SLAB)
        elu(f84p[:], fk[1][:], 128, SLAB)
        elu(f85p[0][:], fk[2][:], 128, SLAB)
        elu(f85p[1][:], fk[3][:], 128, SLAB)

        # x = ELU(w9 @ f): M-tiles sequential to cap PSUM use
        sl = slice(s0, s0 + SLAB)
        for mt, (m0, mn) in enumerate(MSPLIT):
            xp = psX.tile([mn, SLAB], F32, tag="xp", name="xp")
            for kt in range(4):
                nc.tensor.matmul(xp[:], w9cs[0:KSPLIT[kt][1], kt, m0:m0 + mn],
                                 fk[kt][:], start=(kt == 0), stop=(kt == 3))
            elu(xp[:], xg[0:mn, mt, sl], mn, SLAB)
            pump(1)
        # odd-shifted copy, reading only already-written xg
        start = s0 - 1 if s > 0 else 0
        nc.vector.tensor_copy(out=xo[:, :, start:s0 + SLAB - 1],
                              in_=xg[:, :, start + 1:s0 + SLAB])

        # activate any bands whose x pixels are now all written
        while band_q and band_q[0][2] == s:
            r0, nr, _ = band_q.pop(0)
            active.append(emit_band(r0, nr))

    # drain remaining band work (last band has no following slab)
    pump(10 ** 6)


_NC_CACHE = {}
LAST_RESULT = None


def _build_nc():
    if "nc" in _NC_CACHE:
        return _NC_CACHE["nc"]
    nc = bacc.Bacc()
    io = {
        "c6": nc.declare_dram_parameter("c6", [4096, NPX], F32, isOutput=False),
        "c5": nc.declare_dram_parameter("c5", [1024, NPX], F32, isOutput=False),
        "c4": nc.declare_dram_parameter("c4", [512, NPX], F32, isOutput=False),
        "wc": nc.declare_dram_parameter("wc", [128, 9472], BF16, isOutput=False),
        "w9c": nc.declare_dram_parameter("w9c", [128, 4 * 448], BF16, isOutput=False),
        "out": nc.declare_dram_parameter("out", [34, NPAIR], F32, isOutput=True),
    }
    with tile.TileContext(nc) as tc:
        with ExitStack() as ctx:
            _emit(ctx, tc, io)
    nc.finalize()   # Bacc.compile(): 1-wait legalization + event semaphores
    _NC_CACHE["nc"] = nc
    return nc


def _expected_indices():
    full = np.reshape(np.arange(0, 56 * 56, dtype=np.int64), (56, 56))
    ind_from = np.reshape(full[:-4, 4:-4], [-1])
    tos = []
    for dy, dx in OFFS:
        tos.append(np.reshape(full[dy:dy + 52, 4 + dx:4 + dx + 48], [-1]))
    return ind_from, np.concatenate(tos, axis=0)


def _maybe_install_trace_hook():
    import os
    if not os.environ.get("BASS_TRACE"):
        return
    import sys
    import types
    try:
        import antenv.axon_hooks  # noqa: F401
        return
    except ImportError:
        pass
    try:
        from trn_agent_boot.trn_boot import _ntff_profile_via_ctypes
        hook = _ntff_profile_via_ctypes('/opt/axon/libaxon_pjrt.so')
    except Exception:
        hook = None
    import antenv
    mod = types.ModuleType("antenv.axon_hooks")
    mod.get_axon_ntff_profile_hook = lambda: hook
    mod.set_axon_ntff_profile_hook = lambda h: None
    sys.modules["antenv.axon_hooks"] = mod
    antenv.axon_hooks = mod


def kernel(conv4, conv5, conv6, w83, w84, w85, w9, ind_from, ind_to):
    import ml_dtypes
    conv4 = np.asarray(conv4, dtype=np.float32)
    conv5 = np.asarray(conv5, dtype=np.float32)
    conv6 = np.asarray(conv6, dtype=np.float32)
    ef, et = _expected_indices()
    assert np.array_equal(np.asarray(ind_from), ef), "unexpected ind_from"
    assert np.array_equal(np.asarray(ind_to), et), "unexpected ind_to"

    def warrange(w, ktiles):
        # w [M, K] -> lhsT tiles layout [128, ktiles*M]
        wt = np.asarray(w, np.float32).T            # [K, M]
        K, M = wt.shape
        return np.ascontiguousarray(
            wt.reshape(ktiles, 128, M).transpose(1, 0, 2).reshape(128, ktiles * M))

    wc = np.ascontiguousarray(np.concatenate(
        [warrange(w85, 32), warrange(w84, 8), warrange(w83, 4)],
        axis=1)).astype(ml_dtypes.bfloat16)
    w9t_f = np.asarray(w9, np.float32).T          # [448 in, 448 out]
    w9c = np.zeros((128, 4, 448), np.float32)
    ks = [(0, 64), (64, 128), (192, 128), (320, 128)]
    for i, (k0, kn) in enumerate(ks):
        w9c[0:kn, i, :] = w9t_f[k0:k0 + kn, :]
    w9c = np.ascontiguousarray(w9c.reshape(128, 4 * 448)).astype(
        ml_dtypes.bfloat16)

    in_maps = []
    for core in range(N_CORES):
        b, half = core // 2, core % 2
        r0 = 0 if half == 0 else 26
        in_maps.append({
            "c6": np.ascontiguousarray(
                conv6[b, :, r0:r0 + ROWS, :].reshape(4096, NPX)),
            "c5": np.ascontiguousarray(
                conv5[b, :, r0:r0 + ROWS, :].reshape(1024, NPX)),
            "c4": np.ascontiguousarray(
                conv4[b, :, r0:r0 + ROWS, :].reshape(512, NPX)),
            "wc": wc, "w9c": w9c,
        })

    _maybe_install_trace_hook()
    nc = _build_nc()
    res = run_bass_kernel_spmd(nc, in_maps, list(range(N_CORES)))
    global LAST_RESULT
    LAST_RESULT = res

    aff = np.empty((4, 34, 2496), np.float32)
    for core in range(N_CORES):
        b, half = core // 2, core % 2
        aff[b, :, half * NPAIR:(half + 1) * NPAIR] = res.results[core]["out"]
    return aff
